# revision 26
# baseline (speedup 1.0000x reference)
"""GQA attention (B=4,S=2048,D=2048,H=16,KH=4) + RoPE + causal mask on 8 trn2 cores.

Sharding: 8 cores = 4 batches x 2 head-groups. Head groups are KV-ALIGNED:
core group g owns the 8 q-heads whose kv head (h%4) is in {2g, 2g+1}, so each
core only computes K/V projections for its 2 kv heads (no duplication across
the pair). Host sums the two partial outputs per batch (wo row-split).

Per-core pipeline (fp16 matmuls, fp32 accumulate/softmax):
  x fp16 (host pre-transposed) -> xT [d, s] resident
  K/V projections (2 kv heads) -> rope(K) -> kT [hd, 2, s] fp16;
  V [s128, kb, 2, hd|1] fp16
  per head: Q proj -> rope -> qT [hd, 2048]
  per q-chunk qc (512 wide), causal path:
    for kb <= 4qc+3: scoresT = kT-block^T @ qT-chunk, rhs stream trimmed to
      cols >= (kb-4qc)*128 (128-block causal trim); exp via ACT (scale+bias)
    diagonal 128x128 blocks: probs *= lower-tri 0/1 mask on DVE (exp cannot
      overflow fp16: |score*scale| stays small, bias -4)
    AV per q-subblock qs: accumulate kb <= 4qc+qs only:
      y[q, hd|sum] = sum_kb probsT_kb^T @ [V|1]; normalize; PE-transpose
  out_partial[q, dm] = sum_{local h} yT_h^T @ wo_h -> f32 (host adds pairs)

General (non-causal) path computes and masks every block additively
pre-exp (mask accumulated onto scores via identity matmul).
"""
import math

import numpy as np

B, S, D = 4, 2048, 2048
H, KH, HD = 16, 4, 128
HL = 8                   # heads per core
KVL = 2                  # kv heads per core
DC = D // 128            # contraction chunks
NKB = S // 128           # key blocks
NQC = S // 512           # q chunks
NCORES = 8
SCALE = 1.0 / math.sqrt(HD)
EXP_BIAS = -4.0

# core group g owns q-heads with (h % 4) in {2g, 2g+1}; ascending order gives
# local head i -> kv-local index i % 2 uniformly across both groups.
HEAD_GROUPS = [[h for h in range(H) if (h % KH) // KVL == g] for g in range(2)]

_cache = {}


def _build(causal: bool):
    import concourse.bacc as bacc
    import concourse.tile as tile
    import concourse.mybir as mybir

    f16, f32 = mybir.dt.float16, mybir.dt.float32
    Act = mybir.ActivationFunctionType

    nc = bacc.Bacc("TRN2", target_bir_lowering=False, debug=False,
                   num_devices=NCORES)

    # all host-side tensors are pre-swizzled to the exact SBUF layout so
    # every DMA moves large contiguous lines (>=1KB) on both sides
    xt = nc.dram_tensor("xt", [4, 128, DC * 512], f16, kind="ExternalInput").ap()
    wqg = nc.dram_tensor("wqg", [HL, 128, DC * HD], f16,
                         kind="ExternalInput").ap()
    wkg = nc.dram_tensor("wkg", [128, DC * KVL * HD], f16,
                         kind="ExternalInput").ap()
    wvg = nc.dram_tensor("wvg", [128, DC * KVL * HD], f16,
                         kind="ExternalInput").ap()
    wog = nc.dram_tensor("wog", [128, HL * 4 * 512], f16,
                         kind="ExternalInput").ap()
    # causal: [128,128] lower-tri 0/1 multiplicative mask (post-exp);
    # general: fp16 additive mask in pre-scale units [p, kb, q]
    mshape = [128, 128] if causal else [128, NKB, S]
    maskt = nc.dram_tensor("maskt", mshape, f16, kind="ExternalInput").ap()
    c2 = nc.dram_tensor("c2", [128, S], f16, kind="ExternalInput").ap()
    s2 = nc.dram_tensor("s2", [128, S], f16, kind="ExternalInput").ap()
    swp = nc.dram_tensor("swp", [128, 128], f16, kind="ExternalInput").ap()
    ident = nc.dram_tensor("ident", [128, 128], f16, kind="ExternalInput").ap()
    outp = nc.dram_tensor("outp", [S, D], f16, kind="ExternalOutput").ap()

    with tile.TileContext(nc) as tc:
        with tc.tile_pool(name="const", bufs=1) as constp, \
             tc.tile_pool(name="resid", bufs=1) as resid, \
             tc.tile_pool(name="psA", bufs=1, space="PSUM") as psA:
            swpt = constp.tile([128, 128], f16)
            identt = constp.tile([128, 128], f16)
            bias_t = constp.tile([128, 1], f32)
            # small consts go on the vector DMA queue: the sync queue issues
            # serially (~650ns each) and must not delay wk/x at startup
            nc.gpsimd.dma_start(out=swpt, in_=swp)
            nc.gpsimd.dma_start(out=identt, in_=ident)
            nc.vector.memset(bias_t, EXP_BIAS)

            kT = resid.tile([128, KVL, S], f16)            # [hd, kvl, s]
            V = resid.tile([128, NKB, KVL, HD + 1], f16)   # [s128, kb, kvl, hd|1]
            qTs = resid.tile([128, HL, S], f16)            # [hd, h, s]
            for kb in range(NKB):
                nc.vector.memset(V[:, kb, :, HD:HD + 1], 1.0)

            with tc.tile_pool(name="p_x", bufs=1) as p_x:
                xT = p_x.tile([128, 4, DC, 512], f16)     # [d128, sc, dc, s']
                c2t = p_x.tile([128, S], f16)
                s2t = p_x.tile([128, S], f16)
                nc.gpsimd.dma_start(out=c2t, in_=c2)
                nc.gpsimd.dma_start(out=s2t, in_=s2)

                def rope_evict(pP, out_ap, off, ncols, tag):
                    """out = pP*c2 + (SWP @ fp16(pP))*s2, table cols [off, off+ncols)."""
                    psb = p_x.tile([128, 512], f16, name=f"psb_{tag}", tag="psb",
                                   bufs=2)
                    nc.scalar.copy(out=psb[:, 0:ncols], in_=pP)
                    pSw = psA.tile([128, 512], f32, name=f"pSw_{tag}", tag="aux",
                                   bufs=2)
                    nc.tensor.matmul(pSw[:, 0:ncols], swpt, psb[:, 0:ncols],
                                     start=True, stop=True)
                    m1 = p_x.tile([128, 512], f32, name=f"m1_{tag}", tag="m1", bufs=2)
                    m2 = p_x.tile([128, 512], f32, name=f"m2_{tag}", tag="m2", bufs=2)
                    nc.vector.tensor_mul(m1[:, 0:ncols], pP, c2t[:, off:off + ncols])
                    nc.vector.tensor_mul(m2[:, 0:ncols], pSw[:, 0:ncols],
                                         s2t[:, off:off + ncols])
                    nc.gpsimd.tensor_add(out_ap, m1[:, 0:ncols], m2[:, 0:ncols])

                # ---- Phase 1: K/V projections (2 kv heads only) ----
                with tc.tile_pool(name="p_kv", bufs=1) as p_kv:
                    wkt = p_kv.tile([128, DC, KVL * HD], f16)
                    wvt = p_kv.tile([128, DC, KVL * HD], f16)
                    # x arrives pre-transposed and pre-swizzled from the
                    # host. wk first
                    # (one 1MB DMA), then chunk-0 of x in dc-pair pieces so
                    # the first K projection chain starts after ~1.5MB.
                    nc.sync.dma_start(
                        out=wkt, in_=wkg.rearrange("p (c n) -> p c n",
                                                   n=KVL * HD))
                    for dp in range(0, DC, 2):
                        nc.sync.dma_start(
                            out=xT[:, 0, dp:dp + 2, :],
                            in_=xt[0, :, dp * 512:(dp + 2) * 512].rearrange(
                                "p (c n) -> p c n", n=512))
                    nc.sync.dma_start(
                        out=wvt, in_=wvg.rearrange("p (c n) -> p c n",
                                                   n=KVL * HD))
                    for sc in range(1, 4):
                        for dp in range(0, DC, 2):
                            nc.sync.dma_start(
                                out=xT[:, sc, dp:dp + 2, :],
                                in_=xt[sc, :, dp * 512:(dp + 2) * 512].rearrange(
                                    "p (c n) -> p c n", n=512))
                    for sc in range(4):
                        cs = slice(sc * 512, (sc + 1) * 512)
                        for kv in range(KVL):
                            kP = psA.tile([128, 512], f32, name=f"kP{sc}_{kv}",
                                          tag="big", bufs=4)
                            for dc in range(DC):
                                nc.tensor.matmul(kP, wkt[:, dc, kv * HD:(kv + 1) * HD],
                                                 xT[:, sc, dc, :], start=(dc == 0),
                                                 stop=(dc == DC - 1))
                            rope_evict(kP, kT[:, kv, cs], sc * 512, 512, f"k{sc}_{kv}")
                        for sb in range(4):
                            kb = sc * 4 + sb
                            vP = psA.tile([128, 512], f32, name=f"vP{kb}", tag="big",
                                          bufs=4)
                            for dc in range(DC):
                                nc.tensor.matmul(
                                    vP[:, 0:KVL * HD],
                                    xT[:, sc, dc, sb * 128:(sb + 1) * 128],
                                    wvt[:, dc, :], start=(dc == 0),
                                    stop=(dc == DC - 1))
                            nc.scalar.copy(
                                out=V[:, kb, :, 0:HD],
                                in_=vP[:, 0:KVL * HD].rearrange(
                                    "p (kv h) -> p kv h", kv=KVL))

                # ---- Phase 2: all Q projections + rope ----
                for h in range(HL):
                    wqt = p_x.tile([128, DC, HD], f16, name=f"wq{h}", tag="wq", bufs=2)
                    nc.sync.dma_start(
                        out=wqt,
                        in_=wqg[h].rearrange("p (c n) -> p c n", n=HD))
                    for qc in range(NQC):
                        qP = psA.tile([128, 512], f32, name=f"qP{h}_{qc}", tag="big",
                                      bufs=4)
                        for dc in range(DC):
                            nc.tensor.matmul(qP, wqt[:, dc, :],
                                             xT[:, qc, dc, :],
                                             start=(dc == 0), stop=(dc == DC - 1))
                        rope_evict(qP, qTs[:, h, qc * 512:(qc + 1) * 512],
                                   qc * 512, 512, f"q{h}_{qc}")

            # ---- Phase 3: attention; Phase 4: output projection ----
            with tc.tile_pool(name="p_att", bufs=1) as ph, \
                 tc.tile_pool(name="p_4", bufs=1) as p4:
                if causal:
                    trit = ph.tile([128, 128], f16)
                    nc.sync.dma_start(out=trit, in_=maskt)
                wot = p4.tile([128, HL, 4, 512], f16)  # [hd128, h, dmc, dm]
                nc.sync.dma_start(
                    out=wot,
                    in_=wog.rearrange("p (c m n) -> p c m n", m=4, n=512))

                def op_chain(qc, yTsb, qsl, dmc):
                    # one output-projection accumulation chain (no ACT dep:
                    # interleaved between scores and AV to fill the PE while
                    # the scalar engine catches up on exps)
                    qs = qc * 4 + qsl
                    oP = psA.tile([128, 512], f32, name=f"oP{qs}_{dmc}",
                                  tag="big", bufs=4)
                    for h in range(HL):
                        nc.tensor.matmul(
                            oP, yTsb[:, h, qsl * 128:(qsl + 1) * 128],
                            wot[:, h, dmc, :],
                            start=(h == 0), stop=(h == HL - 1))
                    osb = p4.tile([128, 512], f16, name=f"osb{qs}_{dmc}",
                                  tag="osb", bufs=2)
                    nc.vector.tensor_copy(out=osb, in_=oP)
                    nc.sync.dma_start(
                        out=outp[qs * 128:(qs + 1) * 128,
                                 dmc * 512:(dmc + 1) * 512],
                        in_=osb)

                pending = None
                for qc in range(NQC):
                    yTsb = p4.tile([128, HL, 512], f16, name=f"yTsb{qc}",
                                   tag="yTsb", bufs=3)
                    mqc = None
                    if not causal:
                        mqc = ph.tile([128, NKB, 512], f16, name=f"mqc{qc}",
                                      tag="mqc", bufs=2)
                        nc.sync.dma_start(out=mqc,
                                          in_=maskt[:, :, qc * 512:(qc + 1) * 512])
                    for h in range(HL):
                        kv = h % KVL
                        nkbs = 4 * qc + 4 if causal else NKB
                        probs = ph.tile([128, 16, 512], f16, name=f"pr{h}_{qc}",
                                        tag="probs", bufs=4)
                        for kb in range(nkbs):
                            # causal 128-block trim: key block kb only attends
                            # q columns >= (kb - 4qc)*128 within this chunk
                            d = max(0, kb - 4 * qc) if causal else 0
                            cs = slice(d * 128, 512)
                            qs_ap = qTs[:, h, qc * 512 + d * 128:(qc + 1) * 512]
                            sc_ps = psA.tile([128, 512], f32, name=f"sc{h}_{qc}_{kb}",
                                             tag="big", bufs=4)
                            if causal:
                                nc.tensor.matmul(sc_ps[:, cs],
                                                 kT[:, kv, kb * 128:(kb + 1) * 128],
                                                 qs_ap, start=True, stop=True)
                            else:
                                nc.tensor.matmul(sc_ps, kT[:, kv, kb * 128:(kb + 1) * 128],
                                                 qs_ap, start=True, stop=False)
                                # accumulate the additive mask on the PE
                                nc.tensor.matmul(sc_ps, identt, mqc[:, kb, :],
                                                 start=False, stop=True)
                            nc.scalar.activation(out=probs[:, kb, cs], in_=sc_ps[:, cs],
                                                 func=Act.Exp, bias=bias_t,
                                                 scale=SCALE)
                        if pending is not None:
                            pqc, pyT = pending
                            for j in range(2):
                                ci = h * 2 + j
                                op_chain(pqc, pyT, ci // 4, ci % 4)
                        if causal:
                            # diagonal 128x128 blocks: zero strictly-upper part
                            for qs in range(4):
                                kb = 4 * qc + qs
                                blk = probs[:, kb, qs * 128:(qs + 1) * 128]
                                nc.vector.tensor_mul(blk, blk, trit)
                        ysbs = []
                        for qs in range(4):
                            kbs_av = 4 * qc + qs + 1 if causal else NKB
                            yP = psA.tile([128, HD + 1], f32, name=f"yP{h}_{qc}_{qs}",
                                          tag="yP", bufs=2)
                            for kb in range(kbs_av):
                                nc.tensor.matmul(yP,
                                                 probs[:, kb, qs * 128:(qs + 1) * 128],
                                                 V[:, kb, kv, :], start=(kb == 0),
                                                 stop=(kb == kbs_av - 1))
                            rc = ph.tile([128, 1], f32, name=f"rc{h}_{qc}_{qs}",
                                         tag="rc", bufs=2)
                            nc.vector.reciprocal(rc, yP[:, HD:HD + 1])
                            ysb = ph.tile([128, HD], f16, name=f"ysb{h}_{qc}_{qs}",
                                          tag="ysb", bufs=5)
                            nc.vector.tensor_scalar_mul(ysb, yP[:, 0:HD], rc)
                            ysbs.append(ysb)
                        for qs in range(4):
                            yTp = psA.tile([128, 512], f16, name=f"yTp{h}_{qc}_{qs}",
                                           tag="aux", bufs=2)
                            nc.tensor.transpose(yTp[:, 0:128], ysbs[qs], identt)
                            nc.vector.tensor_copy(
                                out=yTsb[:, h, qs * 128:(qs + 1) * 128],
                                in_=yTp[:, 0:128])
                    pending = (qc, yTsb)
                pqc, pyT = pending
                for ci in range(16):
                    op_chain(pqc, pyT, ci // 4, ci % 4)

    nc.compile()
    return nc


def _host_prep(x, wq, wk, wv, wo, freqs_cos, freqs_sin, mask, causal):
    f16 = np.float16
    swp_np = np.zeros((128, 128), dtype=f16)
    idx = np.arange(64)
    swp_np[2 * idx, 2 * idx + 1] = 1.0
    swp_np[2 * idx + 1, 2 * idx] = 1.0
    id_np = np.eye(128, dtype=f16)
    sign = np.tile(np.array([-1.0, 1.0], np.float32), 64)[:, None]
    c2_np = np.ascontiguousarray(np.repeat(freqs_cos.T, 2, axis=0).astype(f16))
    s2_np = np.ascontiguousarray(
        (np.repeat(freqs_sin.T, 2, axis=0) * sign).astype(f16))

    if causal:
        # multiplicative lower-tri 0/1 mask for diagonal 128-blocks (post-exp)
        p = np.arange(128)[:, None]
        q = np.arange(128)[None, :]
        mt = (p <= q).astype(f16)
    else:
        mt = np.clip(mask.astype(np.float64) / SCALE, -1e4, 1e4).astype(f16)
        mt = mt.reshape(NKB, 128, S).transpose(1, 0, 2)
    mt = np.ascontiguousarray(mt)

    shared = {
        "maskt": mt, "c2": c2_np, "s2": s2_np,
        "swp": swp_np, "ident": id_np,
    }
    # pre-swizzle everything to the SBUF layout: DMA lines are then large
    # and contiguous on both sides (packet-efficient)
    xb = [np.ascontiguousarray(
        x[b].T.reshape(DC, 128, 4, 512).transpose(2, 1, 0, 3)
        .reshape(4, 128, DC * 512).astype(f16)) for b in range(B)]
    wqg, wog, wkg, wvg = [], [], [], []
    for g in range(2):
        heads = HEAD_GROUPS[g]
        qcols = np.concatenate([np.arange(h * HD, (h + 1) * HD) for h in heads])
        wqg.append(np.ascontiguousarray(
            wq[:, qcols].reshape(DC, 128, HL, HD).transpose(2, 1, 0, 3)
            .reshape(HL, 128, DC * HD).astype(f16)))
        wog.append(np.ascontiguousarray(
            wo[qcols, :].reshape(HL, 128, 4, 512).transpose(1, 0, 2, 3)
            .reshape(128, HL * 4 * 512).astype(f16)))
        wkg.append(np.ascontiguousarray(
            wk[:, 2 * g * HD:(2 * g + 2) * HD].reshape(DC, 128, KVL * HD)
            .transpose(1, 0, 2).reshape(128, DC * KVL * HD).astype(f16)))
        wvg.append(np.ascontiguousarray(
            wv[:, 2 * g * HD:(2 * g + 2) * HD].reshape(DC, 128, KVL * HD)
            .transpose(1, 0, 2).reshape(128, DC * KVL * HD).astype(f16)))
    in_maps = []
    for core in range(NCORES):
        b, g = core // 2, core % 2
        in_maps.append({"xt": xb[b], "wqg": wqg[g], "wkg": wkg[g],
                        "wvg": wvg[g], "wog": wog[g], **shared})
    return in_maps


def _is_causal(mask: np.ndarray) -> bool:
    if mask.shape != (S, S):
        return False
    iu = np.triu_indices(S, k=1)
    if not np.all(mask[iu] <= -1e8):
        return False
    il = np.tril_indices(S, k=0)
    return bool(np.all(mask[il] == 0.0))


def run(x, wq, wk, wv, wo, freqs_cos, freqs_sin, mask, trace=False):
    from concourse.bass_utils import run_bass_kernel_spmd

    causal = _is_causal(np.asarray(mask))
    key = "causal" if causal else "general"
    if key not in _cache:
        _cache[key] = _build(causal)
    nc = _cache[key]

    in_maps = _host_prep(
        np.asarray(x, np.float32), np.asarray(wq, np.float32),
        np.asarray(wk, np.float32), np.asarray(wv, np.float32),
        np.asarray(wo, np.float32), np.asarray(freqs_cos, np.float32),
        np.asarray(freqs_sin, np.float32), np.asarray(mask, np.float32), causal)

    res = run_bass_kernel_spmd(nc, in_maps, list(range(NCORES)), trace=trace)

    out = np.empty((B, S, D), dtype=np.float32)
    for b in range(B):
        out[b] = (res.results[2 * b]["outp"].astype(np.float32)
                  + res.results[2 * b + 1]["outp"].astype(np.float32))
    return out, res


def kernel(x, wq, wk, wv, wo, freqs_cos, freqs_sin, mask):
    out, _ = run(x, wq, wk, wv, wo, freqs_cos, freqs_sin, mask, trace=False)
    return out
